# revision 1
# baseline (speedup 1.0000x reference)
"""Trainium2 Bass kernel for nn_DynamicConv2d: per-sample dynamic conv.

  feat = x.mean(H,W); h1 = relu(feat@w1+b1); wgen = (h1@w2+b2) -> per-sample
  [COUT, CIN, 3, 3] conv weights; out[s] = conv2d(x[s], wgen[s], pad=1).

Sharding: batch B=32 across 8 cores (4 samples/core), MLP params replicated.

Per-core pipeline (DMA on this sim model is a single serial resource, so the
schedule is built around the in-DMA chain x0 -> x1 -> b2 -> w2):
  - x arrives host-width-padded [4, 64, 128, 130] (zero side cols); loaded as
    two sample-pair images xp [(sp,ci)=128 partitions, 128, 130] bf16
  - feat: chunked reduces (ScalarE accum + DVE) overlapped with the x DMA
  - h1T = matmul(lhsT=w1/(H*W), rhs=feat4) -> Relu+b1 on ScalarE -> bf16
  - wgen: w2 arrives host-permuted OFFSET-major [HID, o, co, ci] in 18
    (o, co-half) slices; per slice 4 chunk matmuls (stationary h1T,
    tile_position packs 4 col-groups), one DVE StreamTranspose lands ci on
    partitions, then strided copies assemble block-diagonal conv weights
    WT2 [(sp,ci), (sp,co), o]; per-offset in-place +b2 adds complete each
    offset's weights as soon as its two slices land
  - conv: per offset o one [K=128, M=128, N<=512] bf16 matmul per pair-tile
    (block-diag stationary covers both samples), 9 offsets accumulate in one
    PSUM bank.  Because w2 is offset-major, pair0's tiles 1-5 run their
    offset-o passes DURING the w2 DMA window (PE is otherwise idle there),
    pulling ~35 passes off the post-wgen critical path.  ScalarE drains;
    DMA out (last groups split small for a short tail).
"""

import sys

for _p in ("/opt/trn_rl_repo",):
    if _p not in sys.path:
        sys.path.insert(0, _p)

from contextlib import ExitStack

import numpy as np

import concourse.bass as bass
import concourse.tile as tile
from concourse import bacc, mybir
from concourse.bass_utils import run_bass_kernel_spmd

F32 = mybir.dt.float32
BF16 = mybir.dt.bfloat16

B, CIN, COUT, K, H, W = 32, 64, 64, 3, 128, 128
NCORES = 8
BSH = B // NCORES          # 4 samples per core
NPAIR = BSH // 2           # 2 sample-pairs per core
HID = 128                  # MLP hidden
JTOT = COUT * CIN * K * K  # 36864
NOFF = K * K               # 9
HW = H * W
WP = W + 2                 # width-padded image

ET = [1, 2, 3, 4, 5]       # pair0 tiles that run early passes (h0 = 4..20)
EO = 6                     # offsets 0..5 run early for those tiles


def build_kernel_body(nc, tc, ctx, aps):
    x_ap = aps["x"]      # [BSH, CIN, H, WP]  (host width-padded)
    w1_ap = aps["w1"]    # [CIN, HID]
    b1_ap = aps["b1"]    # [HID, 1]
    w2_ap = aps["w2"]    # [HID, NOFF, COUT, CIN]  (host offset-major)
    b2_ap = aps["b2"]    # [2*CIN, 2*COUT, NOFF] bf16 block-diag conv layout
    out_ap = aps["out"]  # [BSH, COUT, H, W]

    const = ctx.enter_context(tc.tile_pool(name="const", bufs=1))
    xpool = ctx.enter_context(tc.tile_pool(name="xpool", bufs=2))
    w2pool = ctx.enter_context(tc.tile_pool(name="w2pool", bufs=4))
    tpool = ctx.enter_context(tc.tile_pool(name="tpool", bufs=2))
    wtpool = ctx.enter_context(tc.tile_pool(name="wtpool", bufs=2))
    fpool = ctx.enter_context(tc.tile_pool(name="fpool", bufs=4))
    outp = ctx.enter_context(tc.tile_pool(name="outp", bufs=4))
    mlp_ps = ctx.enter_context(tc.tile_pool(name="mlp_ps", bufs=1, space="PSUM"))
    wg_ps = ctx.enter_context(tc.tile_pool(name="wg_ps", bufs=2, space="PSUM"))
    cv_ps = ctx.enter_context(tc.tile_pool(name="cv_ps", bufs=5, space="PSUM"))

    # ---- zero-fills first: no deps, runs while DMA streams in ----
    wt_tiles = []
    for p in range(NPAIR):
        wt = wtpool.tile([2 * CIN, 2 * COUT, NOFF], BF16, tag="wt", name=f"wt{p}")
        wt_tiles.append(wt)
        nc.vector.memset(wt, 0.0)
    h1T32 = const.tile([HID, 32], BF16)
    nc.vector.memset(h1T32, 0.0)

    # ---- small MLP params on the Activation HWDGE queue ----
    w1_sb = const.tile([CIN, HID], F32)
    nc.scalar.dma_start(out=w1_sb, in_=w1_ap)
    b1_sb = const.tile([HID, 1], F32)
    nc.scalar.dma_start(out=b1_sb, in_=b1_ap)

    # ---- serial in-DMA chain on SP: x pair0, x pair1, b2, w2 slices ----
    NXC = 8  # sub-DMAs per pair
    rows_per = H // NXC
    x2 = x_ap.rearrange("s c h w -> (s c) h w")
    xp_tiles = [None] * NPAIR

    def load_pair(p):
        xp = xpool.tile([2 * CIN, H, WP], BF16, tag="xp", name=f"xp{p}")
        xp_tiles[p] = xp
        for c in range(NXC):
            r0 = c * rows_per
            nc.sync.dma_start(
                out=xp[:, r0 : r0 + rows_per, :],
                in_=x2[2 * p * CIN : (2 * p + 2) * CIN, r0 : r0 + rows_per, :],
            )

    load_pair(0)
    load_pair(1)

    # b2 arrives host-prepped bf16 in block-diagonal conv layout
    # [(sp,ci), (sp,co), o]; added in place per offset once assembled.
    b2T2 = const.tile([2 * CIN, 2 * COUT, NOFF], BF16)
    nc.sync.dma_start(out=b2T2, in_=b2_ap)

    w2sl_tiles = {}
    for o in range(NOFF):
        for half in range(2):
            w2sl = w2pool.tile(
                [HID, 32, CIN], BF16, tag="w2sl", name=f"w2sl{o}_{half}"
            )
            nc.sync.dma_start(
                out=w2sl, in_=w2_ap[:, o, 32 * half : 32 * (half + 1), :]
            )
            w2sl_tiles[(o, half)] = w2sl

    # ---- feat partial reduces chase the x sub-DMAs ----
    fsum4 = const.tile([2 * CIN, BSH], F32)

    def feat_pair(p):
        # per-chunk channel sums on ScalarE (accum_out); keeps DVE free for
        # the wgen StreamTranspose on the critical path
        xp = xp_tiles[p]
        fpart = fpool.tile([2 * CIN, NXC + 1], F32, tag="fpart", name=f"fpart{p}")
        for c in range(NXC):
            r0 = c * rows_per
            if c == NXC - 1:
                hrow = rows_per // 2
                nc.vector.tensor_reduce(
                    out=fpart[:, c : c + 1],
                    in_=xp[:, r0 : r0 + hrow, 1 : W + 1],
                    axis=mybir.AxisListType.XY,
                    op=mybir.AluOpType.add,
                )
                ascr = fpool.tile(
                    [2 * CIN, hrow * W], BF16, tag="ascr", name=f"ascrL{p}"
                )
                nc.scalar.activation(
                    out=ascr,
                    in_=xp[:, r0 + hrow : r0 + rows_per, 1 : W + 1],
                    func=mybir.ActivationFunctionType.Copy,
                    accum_out=fpart[:, c + 1 : c + 2],
                )
            elif c % 2 == 0:
                ascr = fpool.tile(
                    [2 * CIN, rows_per * W], BF16, tag="ascr", name=f"ascr{p}_{c}"
                )
                nc.scalar.activation(
                    out=ascr,
                    in_=xp[:, r0 : r0 + rows_per, 1 : W + 1],
                    func=mybir.ActivationFunctionType.Copy,
                    accum_out=fpart[:, c : c + 1],
                )
            else:
                nc.vector.tensor_reduce(
                    out=fpart[:, c : c + 1],
                    in_=xp[:, r0 : r0 + rows_per, 1 : W + 1],
                    axis=mybir.AxisListType.XY,
                    op=mybir.AluOpType.add,
                )
        nc.vector.tensor_reduce(
            out=fsum4[:, 2 * p : 2 * p + 1],
            in_=fpart,
            axis=mybir.AxisListType.X,
            op=mybir.AluOpType.add,
        )

    feat_pair(0)
    feat_pair(1)

    # ---- MLP (all 4 samples): h1T = relu(w1s.T @ feat4 + b1) -> bf16 ----
    w1s = const.tile([CIN, HID], F32)
    nc.scalar.mul(out=w1s, in_=w1_sb, mul=1.0 / HW)
    feat4 = const.tile([CIN, BSH], F32)
    for p in range(NPAIR):
        nc.vector.tensor_copy(
            out=feat4[:, 2 * p : 2 * p + 1], in_=fsum4[0:CIN, 2 * p : 2 * p + 1]
        )
        nc.vector.tensor_copy(
            out=feat4[:, 2 * p + 1 : 2 * p + 2],
            in_=fsum4[CIN : 2 * CIN, 2 * p : 2 * p + 1],
        )
    h1_ps = mlp_ps.tile([HID, BSH], F32)
    nc.tensor.matmul(out=h1_ps, lhsT=w1s, rhs=feat4, start=True, stop=True)
    nc.scalar.activation(
        out=h1T32[:, 0:BSH],
        in_=h1_ps,
        func=mybir.ActivationFunctionType.Relu,
        bias=b1_sb,
        scale=1.0,
    )

    def badd_off(p, o, eng):
        # in-place +b2 on offset o: all three APs cover the full 128
        # partitions at base 0, the form the hardware's same-base-partition
        # TensorTensor rule allows (off-diagonal rows add b2T2's zeros).
        eng.tensor_tensor(
            out=wt_tiles[p][:, :, o : o + 1],
            in0=wt_tiles[p][:, :, o : o + 1],
            in1=b2T2[:, :, o : o + 1],
            op=mybir.AluOpType.add,
        )

    # early conv psum tiles for pair0 tiles 1..5, held across the w2 window
    ecvp = {}
    for t in ET:
        ecvp[t] = cv_ps.tile([2 * CIN, 4 * W], F32, tag="cvp", name=f"cvpE{t}")

    def conv_pass(cvp, wt, xr, h0, rows, o, start, stop):
        dy, dx = o // 3, o % 3
        h_lo = max(h0, 1 - dy)
        h_hi = min(h0 + rows, H + 1 - dy)
        if h_hi <= h_lo:
            return False
        nc.tensor.matmul(
            out=cvp[:, (h_lo - h0) * W : (h_hi - h0) * W],
            lhsT=wt[:, :, o],
            rhs=xr[:, h_lo + dy - 1 : h_hi + dy - 1, dx : dx + W],
            start=start,
            stop=stop,
        )
        return True

    # ---- wgen: 18 (offset, co-half) slices.  Per slice: 4 chunk matmuls
    # (tile_position packs (co16, ci-half) groups), one StreamTranspose,
    # 8 assembly copies (pair0 on DVE behind its transpose, pair1 on the
    # idle gpsimd), then per-offset +b2 once both halves are in.  Early
    # conv passes for offset o-2 interleave into the PE stream. ----
    for o in range(NOFF):
        for half in range(2):
            w2r = w2sl_tiles[(o, half)]
            wps = wg_ps.tile([2 * CIN, 512], F32, tag="wps", name=f"wps{o}_{half}")
            for g in range(4):  # (co-16-half q, ci-half)
                q, cih = g // 2, g % 2
                nc.tensor.matmul(
                    out=wps[32 * g : 32 * (g + 1), :],
                    lhsT=h1T32,
                    rhs=w2r[:, 16 * q : 16 * (q + 1), 32 * cih : 32 * (cih + 1)],
                    start=True,
                    stop=True,
                    tile_position=(0, 32 * g),
                )
            tmid = tpool.tile(
                [2 * CIN, 512], F32, tag="tmid", name=f"tmid{o}_{half}"
            )
            # per 32x32 block: T[32g + cil, 32c + s] = wps[32g + s, 32c + cil]
            nc.vector.transpose(out=tmid, in_=wps)
            tr = tmid.rearrange("p (co s) -> p co s", co=16, s=32)
            for s in (0, 1, 2, 3):
                pr, sp = s // 2, s % 2
                for q in range(2):
                    cw0 = sp * COUT + 32 * half + 16 * q
                    dst = wt_tiles[pr][
                        sp * CIN : (sp + 1) * CIN, cw0 : cw0 + 16, o : o + 1
                    ]
                    src = tr[64 * q : 64 * (q + 1), :, s : s + 1]
                    if pr == 0:
                        if o >= NOFF - 2:
                            nc.vector.tensor_copy(out=dst, in_=src)
                        else:
                            nc.scalar.copy(out=dst, in_=src)
                    else:
                        nc.gpsimd.tensor_copy(out=dst, in_=src)
            if o == NOFF - 1 and half == 1:
                # offsets 6,7 are complete; run their passes for the early
                # tiles now, before badd0-8 exists -- emitted later they
                # would wait on it (bounding-range hazard on wt0)
                for oo in (6, 7):
                    for t in ET:
                        conv_pass(
                            ecvp[t], wt_tiles[0], xp_tiles[0], 4 * t, 4, oo,
                            start=False, stop=False,
                        )
            if half == 1:
                badd_off(0, o, nc.vector)
                badd_off(1, o, nc.gpsimd)
                oe = o - 2
                if 0 <= oe < EO:
                    for t in ET:
                        conv_pass(
                            ecvp[t], wt_tiles[0], xp_tiles[0], 4 * t, 4, oe,
                            start=(oe == 0), stop=False,
                        )

    # ---- conv ----
    out2 = out_ap.rearrange("s c h w -> (s c) (h w)")
    # center offset first so start=True covers every psum element
    off_order = [4, 0, 1, 2, 3, 5, 6, 7, 8]

    def emit_group(p, h_base, g_rows, tiles_cvp=None, offs=None, start=True):
        g_n = sum(g_rows)
        ost = outp.tile(
            [2 * CIN, g_n * W], F32, tag=f"ost{g_n}",
            name=f"ost{p}_{h_base}",
        )
        ho = 0
        for rows in g_rows:
            h0 = h_base + ho
            if tiles_cvp is not None:
                cvp = tiles_cvp[h0 // 4]
            else:
                cvp = cv_ps.tile(
                    [2 * CIN, 4 * W], F32, tag="cvp", name=f"cvp{p}_{h0}"
                )[:, : rows * W]
            olist = offs if offs is not None else off_order
            live = [
                o for o in olist
                if min(h0 + rows, H + 1 - o // 3) > max(h0, 1 - o // 3)
            ]
            for i, o in enumerate(live):
                conv_pass(
                    cvp, wt_tiles[p], xp_tiles[p], h0, rows, o,
                    start=(start and i == 0), stop=(i == len(live) - 1),
                )
            nc.scalar.copy(out=ost[:, ho * W : (ho + rows) * W], in_=cvp)
            ho += rows
        dma_eng = nc.scalar if g_n == 1 else nc.sync
        dma_eng.dma_start(
            out=out2[
                2 * p * CIN : (2 * p + 2) * CIN,
                h_base * W : (h_base + g_n) * W,
            ],
            in_=ost,
        )

    # pair0: finish the early tiles (offsets 7,8 remain), then the rest
    emit_group(0, 4, [4, 4], tiles_cvp=ecvp, offs=[8], start=False)
    emit_group(0, 12, [4, 4], tiles_cvp=ecvp, offs=[8], start=False)
    emit_group(0, 20, [4], tiles_cvp=ecvp, offs=[8], start=False)
    emit_group(0, 0, [4])
    h_base = 24
    for _ in range(13):
        emit_group(0, h_base, [4, 4])
        h_base += 8

    # pair1: full sweep; final tiles shrink so the last drain+DMA is short
    groups = [[4, 4]] * 15 + [[4, 2], [1], [1]]
    assert sum(r for g in groups for r in g) == H
    h_base = 0
    for g_rows in groups:
        emit_group(1, h_base, g_rows)
        h_base += sum(g_rows)


_CACHE = {}


def build_nc():
    if "nc" in _CACHE:
        return _CACHE["nc"], _CACHE["aps"]
    nc = bacc.Bacc("TRN2", debug=False, num_devices=NCORES)
    aps = {
        "x": nc.dram_tensor("x", [BSH, CIN, H, WP], BF16, kind="ExternalInput").ap(),
        "w1": nc.dram_tensor("w1", [CIN, HID], F32, kind="ExternalInput").ap(),
        "b1": nc.dram_tensor("b1", [HID, 1], F32, kind="ExternalInput").ap(),
        "w2": nc.dram_tensor(
            "w2", [HID, NOFF, COUT, CIN], BF16, kind="ExternalInput"
        ).ap(),
        "b2": nc.dram_tensor(
            "b2", [2 * CIN, 2 * COUT, NOFF], BF16, kind="ExternalInput"
        ).ap(),
        "out": nc.dram_tensor("out", [BSH, COUT, H, W], F32, kind="ExternalOutput").ap(),
    }
    with tile.TileContext(nc) as tc, ExitStack() as ctx:
        build_kernel_body(nc, tc, ctx, aps)
    nc.compile()
    _CACHE["nc"] = nc
    _CACHE["aps"] = aps
    return nc, aps


def make_in_maps(x, w1, b1, w2, b2):
    import ml_dtypes
    x = np.asarray(x, dtype=np.float32)
    xpad = np.zeros((B, CIN, H, WP), dtype=ml_dtypes.bfloat16)
    xpad[:, :, :, 1 : W + 1] = x.astype(ml_dtypes.bfloat16)
    w1 = np.ascontiguousarray(np.asarray(w1, dtype=np.float32))
    b1 = np.ascontiguousarray(np.asarray(b1, dtype=np.float32)).reshape(HID, 1)
    # offset-major permutation: [HID, co, ci, o] -> [HID, o, co, ci]
    w2o = (
        np.asarray(w2, dtype=np.float32)
        .reshape(HID, COUT, CIN, NOFF)
        .transpose(0, 3, 1, 2)
    )
    w2o = np.ascontiguousarray(w2o.astype(ml_dtypes.bfloat16))
    b2v = np.asarray(b2, dtype=np.float32).reshape(COUT, CIN, NOFF)
    b2t = np.zeros((2 * CIN, 2 * COUT, NOFF), dtype=np.float32)
    for sp in range(2):
        b2t[sp * CIN : (sp + 1) * CIN, sp * COUT : (sp + 1) * COUT, :] = (
            b2v.transpose(1, 0, 2)
        )
    b2 = np.ascontiguousarray(b2t.astype(ml_dtypes.bfloat16))
    in_maps = []
    for c in range(NCORES):
        in_maps.append(
            {
                "x": np.ascontiguousarray(xpad[c * BSH : (c + 1) * BSH]),
                "w1": w1,
                "b1": b1,
                "w2": w2o,
                "b2": b2,
            }
        )
    return in_maps


def kernel(x, w1, b1, w2, b2, _trace=False, _results_out=None):
    nc, _ = build_nc()
    in_maps = make_in_maps(x, w1, b1, w2, b2)
    res = run_bass_kernel_spmd(
        nc, in_maps, core_ids=list(range(NCORES)), trace=_trace
    )
    if _results_out is not None:
        _results_out.append(res)
    out = np.concatenate([r["out"] for r in res.results], axis=0)
    return out


if __name__ == "__main__":
    rng = np.random.default_rng(0)
    ins = {
        "x": rng.standard_normal((B, CIN, H, W)).astype(np.float32),
        "w1": (rng.standard_normal((CIN, HID)) * 0.05).astype(np.float32),
        "b1": (rng.standard_normal((HID,)) * 0.05).astype(np.float32),
        "w2": (rng.standard_normal((HID, JTOT)) * 0.05).astype(np.float32),
        "b2": (rng.standard_normal((JTOT,)) * 0.05).astype(np.float32),
    }
    out = kernel(**ins)
    print("out", out.shape, out.dtype, np.abs(out).mean())



# revision 3
# speedup vs baseline: 1.0432x; 1.0432x over previous
"""Trainium2 Bass kernel for nn_DynamicConv2d: per-sample dynamic conv.

  feat = x.mean(H,W); h1 = relu(feat@w1+b1); wgen = (h1@w2+b2) -> per-sample
  [COUT, CIN, 3, 3] conv weights; out[s] = conv2d(x[s], wgen[s], pad=1).

Sharding: batch B=32 across 8 cores (4 samples/core), MLP params replicated.

v1 "X-stationary" scheme (cost model charges matmuls by out-free-size only,
so the win comes from putting 128 useful contraction lanes AND 128 useful
output partitions to work per streamed column):

  - per sample a dup tile [128p, 130, 130] bf16: parts 0-63 = x zero-padded
    (host-padded), parts 64-127 = the same image shifted one column left
    (DVE copy).  A [128, 128] row window of this tile is the matmul
    STATIONARY: lower half covers offset (dy,0), upper half (dy,1).
  - conv: per output row h, 6 matmuls accumulate psum[w=128, co=64]:
      3 "pair" passes  (dy,0)+(dy,1), contraction 128, lhsT=dup[:,h+dy,0:128]
      3 "single" passes (dy,2),       contraction 64,  lhsT=dup[0:64,h+dy,2:130]
    moving = per-sample weight tiles wp[s][dy] [128,64] / ws[s][dy] [64,64].
    Total conv PE rows: 4*128*6*64 = 196k vs 295k for the block-diag scheme.
  - out psum is [w, co]; staged bf16 and DMA'd to HBM [s, w, h, co] (h,co
    contiguous => 2KB descriptors, no small-elem penalty); host transposes
    back to [s, co, h, w] for free.
  - wgen: same 18-slice pipeline as before (4 tile-position-packed matmuls +
    DVE StreamTranspose per slice), but assembled into per-sample per-dy
    moving tiles; b2 added per completed tile.
  - feat: chunked reduces chasing the x DMA, spread over ACT/DVE/Pool so the
    last sample's feat closes ~2us after its DMA.
"""

import sys

for _p in ("/opt/trn_rl_repo",):
    if _p not in sys.path:
        sys.path.insert(0, _p)

from contextlib import ExitStack

import numpy as np

import concourse.bass as bass
import concourse.tile as tile
from concourse import bacc, mybir
from concourse.bass_utils import run_bass_kernel_spmd

F32 = mybir.dt.float32
BF16 = mybir.dt.bfloat16

B, CIN, COUT, K, H, W = 32, 64, 64, 3, 128, 128
NCORES = 8
BSH = B // NCORES          # 4 samples per core
HID = 128                  # MLP hidden
JTOT = COUT * CIN * K * K  # 36864
NOFF = K * K               # 9
HP = H + 2                 # height-padded image rows
WP2 = W + 2                # width-padded image cols
HW = H * W

# w2 slice order: pair offsets (dx 0/1) first, singles (dx=2) last, so the
# pair weight tiles complete early (enables conv pre-run in later versions).
OFF_ORDER = [(0, 0), (0, 1), (1, 0), (1, 1), (2, 0), (2, 1), (0, 2), (1, 2), (2, 2)]

NXC = 5                    # x sub-DMAs per sample (130 rows = 5 x 26)
XROWS = HP // NXC


def build_kernel_body(nc, tc, ctx, aps):
    x_ap = aps["x"]      # [BSH, CIN, HP, WP2] bf16 (host zero-padded)
    w1_ap = aps["w1"]    # [CIN, HID] f32
    b1_ap = aps["b1"]    # [HID, 1] f32
    w2_ap = aps["w2"]    # [HID, NOFF, COUT, CIN] bf16 (host OFF_ORDER-major)
    b2p_ap = aps["b2p"]  # [2*CIN, 3, COUT] bf16: [64*dx+ci, dy, co]
    b2s_ap = aps["b2s"]  # [CIN, 3, COUT] bf16: [ci, dy, co] for dx=2
    out_ap = aps["out"]  # [BSH, W, H, COUT] bf16 (w-major; host untransposes)

    const = ctx.enter_context(tc.tile_pool(name="const", bufs=1))
    dpool = ctx.enter_context(tc.tile_pool(name="dpool", bufs=1))
    w2pool = ctx.enter_context(tc.tile_pool(name="w2pool", bufs=3))
    tpool = ctx.enter_context(tc.tile_pool(name="tpool", bufs=2))
    fpool = ctx.enter_context(tc.tile_pool(name="fpool", bufs=4))
    outp = ctx.enter_context(tc.tile_pool(name="outp", bufs=3))
    mlp_ps = ctx.enter_context(tc.tile_pool(name="mlp_ps", bufs=1, space="PSUM"))
    wg_ps = ctx.enter_context(tc.tile_pool(name="wg_ps", bufs=2, space="PSUM"))
    cv_ps = ctx.enter_context(tc.tile_pool(name="cv_ps", bufs=4, space="PSUM"))

    # ---- tiny zero-init + small params (ACT HWDGE queue) ----
    h1T32 = const.tile([HID, 32], BF16)
    nc.vector.memset(h1T32, 0.0)
    w1_sb = const.tile([CIN, HID], F32)
    nc.scalar.dma_start(out=w1_sb, in_=w1_ap)
    b1_sb = const.tile([HID, 1], F32)
    nc.scalar.dma_start(out=b1_sb, in_=b1_ap)
    b2p_sb = const.tile([2 * CIN, 3, COUT], BF16)
    nc.scalar.dma_start(out=b2p_sb, in_=b2p_ap)
    b2s_sb = const.tile([CIN, 3, COUT], BF16)
    nc.scalar.dma_start(out=b2s_sb, in_=b2s_ap)

    # ---- x DMA chain (SP) + chasing dup copies (DVE) + feat reduces ----
    dup = []
    for s in range(BSH):
        dup.append(dpool.tile([2 * CIN, HP, WP2], BF16, name=f"dup{s}"))
    fsum = const.tile([CIN, BSH], F32)

    for s in range(BSH):
        t = dup[s]
        fpart = fpool.tile([CIN, NXC + 1], F32, tag="fpart", name=f"fpart{s}")
        for c in range(NXC):
            r0, r1 = c * XROWS, (c + 1) * XROWS
            nc.sync.dma_start(out=t[0:CIN, r0:r1, :], in_=x_ap[s, :, r0:r1, :])
            # dup: upper half = image shifted one column left (col 129 of the
            # upper half is never read by any stationary window)
            nc.vector.tensor_copy(
                out=t[CIN : 2 * CIN, r0:r1, 0 : WP2 - 1], in_=t[0:CIN, r0:r1, 1:WP2]
            )
            # feat partial sums chase the DMA; engines split so the last
            # chunk's reduce closes fast
            if c in (0, 1, 3):
                ascr = fpool.tile(
                    [CIN, XROWS * WP2], BF16, tag="ascr", name=f"ascr{s}_{c}"
                )
                nc.scalar.activation(
                    out=ascr,
                    in_=t[0:CIN, r0:r1, :],
                    func=mybir.ActivationFunctionType.Copy,
                    accum_out=fpart[:, c : c + 1],
                )
            elif c == 2:
                nc.vector.tensor_reduce(
                    out=fpart[:, c : c + 1],
                    in_=t[0:CIN, r0:r1, :],
                    axis=mybir.AxisListType.XY,
                    op=mybir.AluOpType.add,
                )
            else:  # last chunk: split ACT / DVE halves
                rm = r0 + XROWS // 2
                ascr = fpool.tile(
                    [CIN, (rm - r0) * WP2], BF16, tag="ascr", name=f"ascrL{s}"
                )
                nc.scalar.activation(
                    out=ascr,
                    in_=t[0:CIN, r0:rm, :],
                    func=mybir.ActivationFunctionType.Copy,
                    accum_out=fpart[:, c : c + 1],
                )
                nc.vector.tensor_reduce(
                    out=fpart[:, c + 1 : c + 2],
                    in_=t[0:CIN, rm:r1, :],
                    axis=mybir.AxisListType.XY,
                    op=mybir.AluOpType.add,
                )
        nc.vector.tensor_reduce(
            out=fsum[:, s : s + 1],
            in_=fpart,
            axis=mybir.AxisListType.X,
            op=mybir.AluOpType.add,
        )

    # ---- MLP: h1 = relu((feat_sums/HW) @ w1 + b1) for all 4 samples ----
    w1s = const.tile([CIN, HID], F32)
    nc.scalar.mul(out=w1s, in_=w1_sb, mul=1.0 / HW)
    h1_ps = mlp_ps.tile([HID, BSH], F32)
    nc.tensor.matmul(out=h1_ps, lhsT=w1s, rhs=fsum, start=True, stop=True)
    nc.scalar.activation(
        out=h1T32[:, 0:BSH],
        in_=h1_ps,
        func=mybir.ActivationFunctionType.Relu,
        bias=b1_sb,
        scale=1.0,
    )

    # ---- wgen: 18 (offset, co-half) slices -> per-sample moving tiles ----
    wp = [[const.tile([2 * CIN, COUT], BF16, name=f"wp{s}_{dy}") for dy in range(3)]
          for s in range(BSH)]
    ws = [[const.tile([CIN, COUT], BF16, name=f"ws{s}_{dy}") for dy in range(3)]
          for s in range(BSH)]

    for k in range(NOFF):
        dy, dx = OFF_ORDER[k]
        for half in range(2):
            w2sl = w2pool.tile(
                [HID, 32, CIN], BF16, tag="w2sl", name=f"w2sl{k}_{half}"
            )
            nc.sync.dma_start(
                out=w2sl, in_=w2_ap[:, k, 32 * half : 32 * (half + 1), :]
            )
            wps = wg_ps.tile([2 * CIN, 512], F32, tag="wps", name=f"wps{k}_{half}")
            for g in range(4):  # (co-16 q, ci-half)
                q, cih = g // 2, g % 2
                nc.tensor.matmul(
                    out=wps[32 * g : 32 * (g + 1), :],
                    lhsT=h1T32,
                    rhs=w2sl[:, 16 * q : 16 * (q + 1), 32 * cih : 32 * (cih + 1)],
                    start=True,
                    stop=True,
                    tile_position=(0, 32 * g),
                )
            tmid = tpool.tile([2 * CIN, 512], F32, tag="tmid", name=f"tmid{k}_{half}")
            nc.vector.transpose(out=tmid, in_=wps)
            tr = tmid.rearrange("p (co s) -> p co s", co=16, s=32)
            for s in range(BSH):
                for q in range(2):
                    co0 = 32 * half + 16 * q
                    src = tr[64 * q : 64 * (q + 1), :, s : s + 1]
                    if dx < 2:
                        dst = wp[s][dy][64 * dx : 64 * dx + CIN, co0 : co0 + 16]
                    else:
                        dst = ws[s][dy][:, co0 : co0 + 16]
                    nc.gpsimd.tensor_copy(out=dst, in_=src)
        # a (s, dy) moving tile is complete once both halves of its last
        # offset landed; add b2 in place (same-base-partition TensorTensor)
        if dx == 1:
            for s in range(BSH):
                nc.vector.tensor_tensor(
                    out=wp[s][dy],
                    in0=wp[s][dy],
                    in1=b2p_sb[:, dy, :],
                    op=mybir.AluOpType.add,
                )
        elif dx == 2:
            for s in range(BSH):
                nc.vector.tensor_tensor(
                    out=ws[s][dy],
                    in0=ws[s][dy],
                    in1=b2s_sb[:, dy, :],
                    op=mybir.AluOpType.add,
                )

    # ---- conv: per (sample, output row) 6 matmuls into psum[w, co] ----
    out_v = out_ap.rearrange("s w h c -> w s (h c)")  # [W, BSH, H*COUT]
    GR = 8                     # rows per psum group (one 2KB bank)
    SG = 2                     # psum groups per staging tile / out DMA

    for s in range(BSH):
        t = dup[s]
        for gg in range(H // (GR * SG)):
            ost = outp.tile([W, GR * SG * COUT], BF16, tag="ost", name=f"ost{s}_{gg}")
            for sub in range(SG):
                g = SG * gg + sub
                pt = cv_ps.tile([W, GR, COUT], F32, tag="pt", name=f"pt{s}_{g}")
                for r in range(GR):
                    h = GR * g + r
                    po = pt[:, r, :]
                    for dy in range(3):
                        nc.tensor.matmul(
                            out=po,
                            lhsT=t[:, h + dy, 0:W],
                            rhs=wp[s][dy],
                            start=(dy == 0),
                            stop=False,
                        )
                    for dy in range(3):
                        nc.tensor.matmul(
                            out=po,
                            lhsT=t[0:CIN, h + dy, 2 : 2 + W],
                            rhs=ws[s][dy],
                            start=False,
                            stop=(dy == 2),
                        )
                nc.scalar.copy(
                    out=ost[:, sub * GR * COUT : (sub + 1) * GR * COUT], in_=pt
                )
            h0 = GR * SG * gg
            nc.sync.dma_start(
                out=out_v[:, s, h0 * COUT : (h0 + GR * SG) * COUT], in_=ost
            )


_CACHE = {}


def build_nc():
    if "nc" in _CACHE:
        return _CACHE["nc"], _CACHE["aps"]
    nc = bacc.Bacc("TRN2", debug=False, num_devices=NCORES)
    aps = {
        "x": nc.dram_tensor("x", [BSH, CIN, HP, WP2], BF16, kind="ExternalInput").ap(),
        "w1": nc.dram_tensor("w1", [CIN, HID], F32, kind="ExternalInput").ap(),
        "b1": nc.dram_tensor("b1", [HID, 1], F32, kind="ExternalInput").ap(),
        "w2": nc.dram_tensor(
            "w2", [HID, NOFF, COUT, CIN], BF16, kind="ExternalInput"
        ).ap(),
        "b2p": nc.dram_tensor("b2p", [2 * CIN, 3, COUT], BF16, kind="ExternalInput").ap(),
        "b2s": nc.dram_tensor("b2s", [CIN, 3, COUT], BF16, kind="ExternalInput").ap(),
        "out": nc.dram_tensor(
            "out", [BSH, W, H, COUT], BF16, kind="ExternalOutput"
        ).ap(),
    }
    with tile.TileContext(nc) as tc, ExitStack() as ctx:
        build_kernel_body(nc, tc, ctx, aps)
    nc.compile()
    _CACHE["nc"] = nc
    _CACHE["aps"] = aps
    return nc, aps


def make_in_maps(x, w1, b1, w2, b2):
    import ml_dtypes

    x = np.asarray(x, dtype=np.float32)
    xpad = np.zeros((B, CIN, HP, WP2), dtype=ml_dtypes.bfloat16)
    xpad[:, :, 1 : H + 1, 1 : W + 1] = x.astype(ml_dtypes.bfloat16)
    w1 = np.ascontiguousarray(np.asarray(w1, dtype=np.float32))
    b1 = np.ascontiguousarray(np.asarray(b1, dtype=np.float32)).reshape(HID, 1)

    # w2 -> [HID, k(OFF_ORDER), co, ci]
    w2r = np.asarray(w2, dtype=np.float32).reshape(HID, COUT, CIN, K, K)
    w2o = w2r.transpose(0, 3, 4, 1, 2).reshape(HID, NOFF, COUT, CIN)
    ko = [3 * dy + dx for (dy, dx) in OFF_ORDER]
    w2o = np.ascontiguousarray(w2o[:, ko].astype(ml_dtypes.bfloat16))

    # b2 -> pair tile [64*dx+ci, dy, co] and single tile [ci, dy, co]
    b2v = np.asarray(b2, dtype=np.float32).reshape(COUT, CIN, K, K)
    b2p = np.zeros((2 * CIN, 3, COUT), dtype=np.float32)
    for dx in range(2):
        b2p[64 * dx : 64 * dx + CIN] = b2v[:, :, :, dx].transpose(1, 2, 0)
    b2s = np.ascontiguousarray(
        b2v[:, :, :, 2].transpose(1, 2, 0).astype(ml_dtypes.bfloat16)
    )
    b2p = np.ascontiguousarray(b2p.astype(ml_dtypes.bfloat16))

    in_maps = []
    for c in range(NCORES):
        in_maps.append(
            {
                "x": np.ascontiguousarray(xpad[c * BSH : (c + 1) * BSH]),
                "w1": w1,
                "b1": b1,
                "w2": w2o,
                "b2p": b2p,
                "b2s": b2s,
            }
        )
    return in_maps


def kernel(x, w1, b1, w2, b2, _trace=False, _results_out=None):
    nc, _ = build_nc()
    in_maps = make_in_maps(x, w1, b1, w2, b2)
    res = run_bass_kernel_spmd(
        nc, in_maps, core_ids=list(range(NCORES)), trace=_trace
    )
    if _results_out is not None:
        _results_out.append(res)
    # out arrives [BSH, W, H, CO] bf16 per core -> [B, CO, H, W] f32
    out = np.concatenate([np.asarray(r["out"]) for r in res.results], axis=0)
    return out.transpose(0, 3, 2, 1).astype(np.float32)


if __name__ == "__main__":
    rng = np.random.default_rng(0)
    ins = {
        "x": rng.standard_normal((B, CIN, H, W)).astype(np.float32),
        "w1": (rng.standard_normal((CIN, HID)) * 0.05).astype(np.float32),
        "b1": (rng.standard_normal((HID,)) * 0.05).astype(np.float32),
        "w2": (rng.standard_normal((HID, JTOT)) * 0.05).astype(np.float32),
        "b2": (rng.standard_normal((JTOT,)) * 0.05).astype(np.float32),
    }
    out = kernel(**ins)
    print("out", out.shape, out.dtype, np.abs(out).mean())
